# revision 3
# baseline (speedup 1.0000x reference)
"""Multi-head GQA attention (RoPE, causal) on 8 TRN2 NeuronCores — v2.

Problem: B=1, S=2048, DIM=2048, 32 Q heads / 8 KV heads, head_dim=64, fp32 in.

Strategy (tensor parallel over heads, no collectives):
  - Core c owns Q heads 4c..4c+3 and KV head c (GQA group == core).
  - Each core computes partial out = attn_c @ woT_c; host sums 8 partials.
  - Scores computed transposed (S^T = K_rot^T.T @ Q_rot^T) so softmax's sum
    runs over the partition axis, obtained free via a ones-column in the AV
    stationary (row 64 of AV output = sum(exp)).
  - v2 changes vs v1 (296us): fully interleaved single-pipeline emission
    (projections / attention / output-projection overlap), HAM prewarm,
    2-head-batched exp on ScalarE, causal mask multiply on GpSimd, bf16
    RoPE on DVE fast modes, reciprocal_approx_fast, per-(hp,j) normalize
    so the wo-projection streams, fewer+bigger DMAs, output DMA on its own
    queue. PSUM plan: proj(2) + scores(4) + av(2) = 8 banks.
"""
import sys

if "/opt/trn_rl_repo" not in sys.path:
    sys.path.insert(0, "/opt/trn_rl_repo")

import numpy as np

import concourse.bass as bass
import concourse.tile as tile
from concourse import bacc, mybir
from concourse.bass_utils import run_bass_kernel_spmd

# ---- problem constants (hardcoded per contract) ----
S = 2048          # sequence length
D = 2048          # model dim
NH = 32           # total Q heads
NKV = 8           # total KV heads
DH = 64           # head dim
NCORES = 8
HQ = NH // NCORES     # 4 Q heads per core
SQC = 512             # sq chunk
SKC = 128             # sk chunk
DC = 128              # d-chunk for projections
NSQ = S // SQC        # 4
NSK = S // SKC        # 16
NDC = D // DC         # 16

F32 = mybir.dt.float32
BF16 = mybir.dt.bfloat16

import os as _os
PREWARM = int(_os.environ.get("PREWARM", "18"))

_PROGRAM_CACHE = {}


def build_program():
    """Build the SPMD Bass program (identical on all 8 cores)."""
    if "nc" in _PROGRAM_CACHE:
        return _PROGRAM_CACHE["nc"]

    nc = bacc.Bacc("TRN2", target_bir_lowering=False, debug=False,
                   num_devices=NCORES)

    xT = nc.dram_tensor("xT", [D, S], BF16, kind="ExternalInput")
    wq_il = nc.dram_tensor("wq_il", [128, NDC, HQ * DH], BF16,
                           kind="ExternalInput")
    wkv_il = nc.dram_tensor("wkv_il", [128, NDC, 2 * DH], BF16,
                            kind="ExternalInput")
    wo_il = nc.dram_tensor("wo_il", [128, 2, D], BF16, kind="ExternalInput")
    cos4 = nc.dram_tensor("cos4", [128, S], BF16, kind="ExternalInput")
    sin4 = nc.dram_tensor("sin4", [128, S], BF16, kind="ExternalInput")
    mask2 = nc.dram_tensor("mask2", [128, 4, 2, SQC], BF16,
                           kind="ExternalInput")
    out = nc.dram_tensor("out", [S, D], BF16, kind="ExternalOutput")

    from concourse.masks import make_identity
    EXP = mybir.ActivationFunctionType.Exp

    with tile.TileContext(nc) as tc:
        with tc.tile_pool(name="const", bufs=1) as cpool, \
             tc.tile_pool(name="work", bufs=2) as wpool, \
             tc.tile_pool(name="ps", bufs=2, space="PSUM") as ps:

            # ---- SBUF-resident constants / weights ----
            xfull = [cpool.tile([128, S], BF16, name=f"xfull{d}")
                     for d in range(NDC)]
            wq_t = cpool.tile([128, NDC, HQ * DH], BF16, name="wq_t")
            wkv_t = cpool.tile([128, NDC, 2 * DH], BF16, name="wkv_t")
            wo_t = cpool.tile([128, 2, D], BF16, name="wo_t")
            cos_t = cpool.tile([128, S], BF16, name="cos_t")
            sin_t = cpool.tile([128, S], BF16, name="sin_t")
            mask_t = cpool.tile([128, 4, 2, SQC], BF16, name="mask_t")
            ident = cpool.tile([128, 128], BF16, name="ident")
            make_identity(nc, ident[:])

            # persistent intermediates
            qrot = [cpool.tile([128, S], BF16, name=f"qrot{t}") for t in range(2)]
            krot = cpool.tile([128, S], BF16, name="krot")
            vaug = cpool.tile([128, NSK, DH + 1], BF16, name="vaug")
            nc.vector.memset(vaug[:, :, DH:DH + 1], 1.0)
            attnT = [cpool.tile([128, S], BF16, name=f"attnT{t}") for t in range(2)]
            zg = cpool.tile([64, SQC], F32, name="zg")
            nc.vector.memset(zg[:], 1.0)
            zr = cpool.tile([64, SQC], F32, name="zr")
            z0 = cpool.tile([1, SQC], F32, name="z0")

            # ---- HAM prewarm: dense dummy matmuls with no DMA deps ----
            scratch = ps.tile([128, 2, SQC], F32, name="warm", tag="st", bufs=2)
            for _ in range(PREWARM):
                nc.tensor.matmul(scratch[:, 0, 0:128], ident[:], ident[:],
                                 start=True, stop=True)

            # ---------------- emission helpers ----------------
            def emit_dmas_pre():
                # ordered for earliest compute start; sync queue is FIFO
                nc.sync.dma_start(wkv_t[:], wkv_il.ap())
                nc.sync.dma_start(wq_t[:, 0:4, :], wq_il[:, 0:4, :])
                nc.sync.dma_start(xfull[0][:, 0:1024], xT[0:128, 0:1024])
                nc.sync.dma_start(xfull[0][:, 1024:2048], xT[0:128, 1024:2048])
                nc.sync.dma_start(xfull[1][:], xT[128:256, :])
                nc.sync.dma_start(cos_t[:], cos4.ap())
                nc.sync.dma_start(sin_t[:], sin4.ap())
                for d in range(2, 6):
                    nc.sync.dma_start(xfull[d][:], xT[d * DC:(d + 1) * DC, :])
                nc.sync.dma_start(wq_t[:, 4:16, :], wq_il[:, 4:16, :])
                nc.sync.dma_start(mask_t[:], mask2.ap())
                for d in range(6, NDC):
                    nc.sync.dma_start(xfull[d][:], xT[d * DC:(d + 1) * DC, :])
                nc.sync.dma_start(wo_t[:], wo_il.ap())

            def emit_A(jp):
                """Projections + RoPE + V-transpose for seq chunk pair
                (j0=2jp, j1=2jp+1), i.e. seq cols [1024jp, 1024jp+1024)."""
                c0 = 1024 * jp          # start col of the pair
                j0, j1 = 2 * jp, 2 * jp + 1
                s0, s1 = j0 * SQC, j1 * SQC

                # --- KV projections (critical path for B) ---
                kv = [ps.tile([128, SQC], F32, name=f"kv{j}", tag="proj")
                      for j in range(2)]
                for d in range(NDC):
                    st_, sp = (d == 0), (d == NDC - 1)
                    nc.tensor.matmul(kv[0][:], wkv_t[:, d, :],
                                     xfull[d][:, s0:s0 + SQC], start=st_, stop=sp)
                    nc.tensor.matmul(kv[1][:], wkv_t[:, d, :],
                                     xfull[d][:, s1:s1 + SQC], start=st_, stop=sp)

                # --- RoPE on K + V evac (bf16, DVE fast modes) ---
                ke = wpool.tile([64, 1024], BF16, name="ke", tag="ke")
                nc.vector.tensor_copy(ke[:, 0:SQC], kv[0][0:64, :])
                nc.vector.tensor_copy(ke[:, SQC:1024], kv[1][0:64, :])
                vtmp = wpool.tile([64, 1024], BF16, name="vtmp", tag="vtmp")
                nc.scalar.copy(vtmp[:, 0:SQC], kv[0][64:128, :])
                nc.scalar.copy(vtmp[:, SQC:1024], kv[1][64:128, :])
                ksw = wpool.tile([64, 1024], BF16, name="ksw", tag="ksw")
                nc.vector.tensor_copy(ksw[0:32, :], ke[32:64, :])
                nc.vector.tensor_copy(ksw[32:64, :], ke[0:32, :])
                nc.vector.tensor_mul(ke[:], ke[:], cos_t[0:64, c0:c0 + 1024])
                nc.vector.tensor_mul(ksw[:], ksw[:], sin_t[0:64, c0:c0 + 1024])
                nc.vector.tensor_add(krot[0:64, c0:c0 + 1024], ke[:], ksw[:])
                nc.vector.tensor_copy(krot[64:128, c0:c0 + 1024],
                                      krot[0:64, c0:c0 + 1024])

                # --- Q projections, half h = heads (2h, 2h+1) -> qrot[h] ---
                for h in range(2):
                    qt = [ps.tile([128, SQC], F32, name=f"qt{h}{j}", tag="proj")
                          for j in range(2)]
                    for d in range(NDC):
                        st_, sp = (d == 0), (d == NDC - 1)
                        nc.tensor.matmul(qt[0][:], wq_t[:, d, 128 * h:128 * h + 128],
                                         xfull[d][:, s0:s0 + SQC], start=st_, stop=sp)
                        nc.tensor.matmul(qt[1][:], wq_t[:, d, 128 * h:128 * h + 128],
                                         xfull[d][:, s1:s1 + SQC], start=st_, stop=sp)
                    qe = wpool.tile([128, 1024], BF16, name="qe", tag="qe")
                    nc.vector.tensor_copy(qe[:, 0:SQC], qt[0][:])
                    nc.vector.tensor_copy(qe[:, SQC:1024], qt[1][:])
                    qsw = wpool.tile([128, 1024], BF16, name="qsw", tag="qsw")
                    for g in range(4):
                        src = 32 * (g ^ 1)
                        nc.vector.tensor_copy(qsw[32 * g:32 * g + 32, :],
                                              qe[src:src + 32, :])
                    nc.vector.tensor_mul(qe[:], qe[:], cos_t[:, c0:c0 + 1024])
                    nc.vector.tensor_mul(qsw[:], qsw[:], sin_t[:, c0:c0 + 1024])
                    nc.vector.tensor_add(qrot[h][:, c0:c0 + 1024], qe[:], qsw[:])

                # --- V transpose: 8 PE transposes -> vaug chunks ---
                tps = [ps.tile([128, SQC], BF16, name=f"tps{j}", tag="proj")
                       for j in range(2)]
                for j in range(2):
                    for b in range(4):
                        i = 4 * (2 * jp + j) + b
                        dst = tps[j][:, 64 * b:64 * b + 64]
                        nc.tensor.transpose(dst, vtmp[:, (4 * j + b) * 128:
                                                      (4 * j + b) * 128 + 128],
                                            ident[0:64, 0:64])
                        nc.vector.tensor_copy(vaug[:, i, 0:DH], dst)

            def emit_B(hp, j):
                """Attention for head pair hp, sq chunk j."""
                s0 = j * SQC
                q = qrot[hp]
                av = [ps.tile([DH + 1, SQC], F32, name=f"av{h}", tag="av")
                      for h in range(2)]
                nsk_j = 4 * j + 4
                for i in range(nsk_j):
                    k0 = i * SKC
                    m = i - 4 * j
                    off = 0 if m < 1 else 128 * m
                    nw = SQC - off
                    st2 = ps.tile([128, 2, SQC], F32, name="st2", tag="st",
                                  bufs=2)
                    for h in range(2):
                        r0 = 64 * h
                        nc.tensor.matmul(st2[:, h, off:SQC],
                                         krot[r0:r0 + 64, k0:k0 + SKC],
                                         q[r0:r0 + 64, s0 + off:s0 + SQC],
                                         start=True, stop=True,
                                         tile_position=(r0, 0))
                    pt = wpool.tile([128, 2, SQC], BF16, name="pt", tag="pt",
                                    bufs=4)
                    nc.scalar.activation(pt[:, :, off:SQC], st2[:, :, off:SQC],
                                         EXP, scale=0.125)
                    if m >= 0:
                        nc.gpsimd.tensor_mul(pt[:, :, off:SQC],
                                             pt[:, :, off:SQC],
                                             mask_t[:, m, :, off:SQC])
                    for h in range(2):
                        nc.tensor.matmul(av[h][:, off:SQC], vaug[:, i, :],
                                         pt[:, h, off:SQC],
                                         start=(i == 0), stop=(i == nsk_j - 1))

                # softmax normalize: 1/Z broadcast, write attnT
                for h in range(2):
                    nc.vector.tensor_copy(zg[32 * h:32 * h + 1, :], av[h][64:65, :])
                nc.vector.reciprocal_approx_fast(zr[0:33, :], zg[0:33, :])
                for h in range(2):
                    if h == 0:
                        zsrc = zr[0:1, :]
                    else:
                        nc.vector.tensor_copy(z0[:], zr[32:33, :])
                        zsrc = z0[:]
                    bc = wpool.tile([64, SQC], F32, name="bc", tag="bc")
                    nc.gpsimd.partition_broadcast(bc[:], zsrc)
                    nc.vector.tensor_mul(attnT[hp][64 * h:64 * h + 64, s0:s0 + SQC],
                                         av[h][0:64, :], bc[:])

            def emit_C(si):
                """Output projection for seq rows [128si, 128si+128)."""
                stage = wpool.tile([128, D], BF16, name="cstage", tag="cstage")
                for oi in range(4):
                    o0 = oi * SQC
                    cps = ps.tile([128, SQC], F32, name="cps", tag="proj")
                    for t in range(2):
                        nc.tensor.matmul(cps[:],
                                         attnT[t][:, si * 128:(si + 1) * 128],
                                         wo_t[:, t, o0:o0 + SQC],
                                         start=(t == 0), stop=(t == 1))
                    nc.vector.tensor_copy(stage[:, o0:o0 + SQC], cps[:])
                nc.gpsimd.dma_start(out[si * 128:(si + 1) * 128, :], stage[:])

            # ---------------- the interleaved pipeline ----------------
            emit_dmas_pre()
            emit_A(0)
            emit_B(0, 0)
            emit_B(0, 1)
            emit_A(1)
            emit_B(0, 2)
            emit_B(0, 3)
            emit_B(1, 0)
            emit_B(1, 1)
            for si in range(0, 4):
                emit_C(si)
            emit_B(1, 2)
            for si in range(4, 8):
                emit_C(si)
            emit_B(1, 3)
            for si in range(8, 16):
                emit_C(si)

    nc.compile()
    _PROGRAM_CACHE["nc"] = nc
    return nc


def prep_in_maps(x, freqs_cos, freqs_sin, wq, wk, wv, wo):
    """Host-side sharding / pre-transposition. Returns list of 8 in_maps."""
    import ml_dtypes
    mmd = ml_dtypes.bfloat16

    x = np.asarray(x, dtype=np.float32)
    freqs_cos = np.asarray(freqs_cos, dtype=np.float32)
    freqs_sin = np.asarray(freqs_sin, dtype=np.float32)
    wq = np.asarray(wq, dtype=np.float32)
    wk = np.asarray(wk, dtype=np.float32)
    wv = np.asarray(wv, dtype=np.float32)
    wo = np.asarray(wo, dtype=np.float32)

    xT = np.ascontiguousarray(x.reshape(S, D).T).astype(mmd)   # [D, S]

    # head-dim permutation: even lanes first, odd lanes second
    perm = np.concatenate([np.arange(0, DH, 2), np.arange(1, DH, 2)])
    wq_h = wq.reshape(NH, DH, D)[:, perm, :]               # [NH, DH, D]
    wk_h = wk.reshape(NKV, DH, D)[:, perm, :]              # [NKV, DH, D]
    wv_h = wv.reshape(NKV, DH, D)                          # not permuted

    # cos rows tiled x4; sin rows: [-sin; +sin] tiled x2 (signs baked in)
    cosT = np.ascontiguousarray(freqs_cos.T)               # [32, S]
    sinT = np.ascontiguousarray(freqs_sin.T)
    cos4 = np.ascontiguousarray(np.tile(cosT, (4, 1))).astype(mmd)
    sin4 = np.ascontiguousarray(
        np.tile(np.concatenate([-sinT, sinT], axis=0), (2, 1))).astype(mmd)

    # causal masks for 4 diagonal offsets, duplicated for the 2 heads of a
    # pair: mask2[p, m, h, f] = f >= p + 128m
    p_idx = np.arange(128)[:, None, None, None]
    m_idx = np.arange(4)[None, :, None, None]
    f_idx = np.arange(SQC)[None, None, None, :]
    mask2 = np.broadcast_to((f_idx >= p_idx + 128 * m_idx),
                            (128, 4, 2, SQC)).astype(mmd)
    mask2 = np.ascontiguousarray(mask2)

    in_maps = []
    for c in range(NCORES):
        wq_c = wq_h[HQ * c:HQ * (c + 1)].reshape(HQ * DH, D)   # [256, D]
        wqT_c = np.ascontiguousarray(wq_c.T).astype(mmd)       # [D, 256]
        wq_int = np.ascontiguousarray(
            wqT_c.reshape(NDC, 128, HQ * DH).transpose(1, 0, 2))
        wkv_c = np.concatenate([wk_h[c], wv_h[c]], axis=0)     # [128, D]
        wkvT_c = np.ascontiguousarray(wkv_c.T).astype(mmd)     # [D, 128]
        wkv_int = np.ascontiguousarray(
            wkvT_c.reshape(NDC, 128, 2 * DH).transpose(1, 0, 2))
        woT_c = np.ascontiguousarray(
            wo[:, HQ * DH * c:HQ * DH * (c + 1)].T).astype(mmd)  # [256, D]
        wo_int = np.ascontiguousarray(
            woT_c.reshape(2, 128, D).transpose(1, 0, 2))
        in_maps.append({
            "xT": xT, "wq_il": wq_int, "wkv_il": wkv_int, "wo_il": wo_int,
            "cos4": cos4, "sin4": sin4, "mask2": mask2,
        })
    return in_maps


def run(inputs, trace=False, trace_cores=None, tmpdir=None):
    """Compile (cached), run on 8 cores, gather. Returns (output, results)."""
    nc = build_program()
    in_maps = prep_in_maps(**inputs)
    res = run_bass_kernel_spmd(nc, in_maps, core_ids=list(range(NCORES)),
                               trace=trace, trace_cores=trace_cores,
                               tmpdir=tmpdir)
    acc = np.zeros((S, D), dtype=np.float32)
    for r in res.results:
        acc += r["out"].astype(np.float32)
    return acc.reshape(1, S, D), res


def kernel(**inputs):
    out, _ = run(inputs)
    return out


# revision 4
# speedup vs baseline: 1.8642x; 1.8642x over previous
"""Multi-head GQA attention (RoPE, causal) on 8 TRN2 NeuronCores — v3.

Problem: B=1, S=2048, DIM=2048, 32 Q heads / 8 KV heads, head_dim=64, fp32 in.

Strategy (tensor parallel over heads, no collectives):
  - Core c owns Q heads 4c..4c+3 and KV head c (GQA group == core).
  - Each core computes partial out = attn_c @ woT_c; host sums 8 partials.
  - Scores computed transposed (S^T = K_rot^T.T @ Q_rot^T) so softmax's sum
    runs over the partition axis, obtained free via a ones-column in the AV
    stationary (row 64 of AV output = sum(exp)).
  - Single interleaved pipeline; phase A(0) runs 6 accumulators in parallel
    so the PE stays dense during the input-DMA window; AV results evacuate
    to SBUF immediately so softmax normalization never blocks the next
    chunk's PSUM reuse; exp is 2-head batched on ScalarE; the wo projection
    (C) streams as soon as both head-pairs normalize a chunk.
  - PSUM plan: proj(2) + st(4) + av(2) = 8 banks; A(0) borrows st's 4.
"""
import sys

if "/opt/trn_rl_repo" not in sys.path:
    sys.path.insert(0, "/opt/trn_rl_repo")

import numpy as np

import concourse.bass as bass
import concourse.tile as tile
from concourse import bacc, mybir
from concourse.bass_utils import run_bass_kernel_spmd

# ---- problem constants (hardcoded per contract) ----
S = 2048          # sequence length
D = 2048          # model dim
NH = 32           # total Q heads
NKV = 8           # total KV heads
DH = 64           # head dim
NCORES = 8
HQ = NH // NCORES     # 4 Q heads per core
SQC = 512             # sq chunk
SKC = 128             # sk chunk
DC = 128              # d-chunk for projections
NSQ = S // SQC        # 4
NSK = S // SKC        # 16
NDC = D // DC         # 16

F32 = mybir.dt.float32
BF16 = mybir.dt.bfloat16

import os as _os
PREWARM = int(_os.environ.get("PREWARM", "18"))

_PROGRAM_CACHE = {}


def build_program():
    """Build the SPMD Bass program (identical on all 8 cores)."""
    if "nc" in _PROGRAM_CACHE:
        return _PROGRAM_CACHE["nc"]

    nc = bacc.Bacc("TRN2", target_bir_lowering=False, debug=False,
                   num_devices=NCORES)

    xT = nc.dram_tensor("xT", [D, S], BF16, kind="ExternalInput")
    wq_il = nc.dram_tensor("wq_il", [128, NDC, HQ * DH], BF16,
                           kind="ExternalInput")
    wkv_il = nc.dram_tensor("wkv_il", [128, NDC, 2 * DH], BF16,
                            kind="ExternalInput")
    wo_il = nc.dram_tensor("wo_il", [128, 2, D], BF16, kind="ExternalInput")
    cos4 = nc.dram_tensor("cos4", [128, S], BF16, kind="ExternalInput")
    sin4 = nc.dram_tensor("sin4", [128, S], BF16, kind="ExternalInput")
    mask2 = nc.dram_tensor("mask2", [128, 4, 2, SQC], BF16,
                           kind="ExternalInput")
    out = nc.dram_tensor("out", [S, D], BF16, kind="ExternalOutput")

    from concourse.masks import make_identity
    EXP = mybir.ActivationFunctionType.Exp

    with tile.TileContext(nc) as tc:
        with tc.tile_pool(name="const", bufs=1) as cpool, \
             tc.tile_pool(name="work", bufs=2) as wpool, \
             tc.tile_pool(name="ps", bufs=2, space="PSUM") as ps:

            # ---- SBUF-resident constants / weights ----
            xfull = [cpool.tile([128, S], BF16, name=f"xfull{d}")
                     for d in range(NDC)]
            wq_t = cpool.tile([128, NDC, HQ * DH], BF16, name="wq_t")
            wkv_t = cpool.tile([128, NDC, 2 * DH], BF16, name="wkv_t")
            wo_t = cpool.tile([128, 2, D], BF16, name="wo_t")
            cos_t = cpool.tile([128, S], BF16, name="cos_t")
            sin_t = cpool.tile([128, S], BF16, name="sin_t")
            mask_t = cpool.tile([128, 4, 2, SQC], BF16, name="mask_t")
            ident = cpool.tile([128, 128], BF16, name="ident")
            make_identity(nc, ident[:])

            # persistent intermediates
            qrot = [cpool.tile([128, S], BF16, name=f"qrot{t}") for t in range(2)]
            krot = cpool.tile([128, S], BF16, name="krot")
            vaug = cpool.tile([128, NSK, DH + 1], BF16, name="vaug")
            nc.vector.memset(vaug[:, :, DH:DH + 1], 1.0)
            attnT = [cpool.tile([128, S], BF16, name=f"attnT{t}") for t in range(2)]
            zg = cpool.tile([64, SQC], F32, name="zg")
            nc.vector.memset(zg[:], 1.0)
            zr = cpool.tile([64, SQC], F32, name="zr")
            z0 = cpool.tile([1, SQC], F32, name="z0")

            # ---- HAM prewarm: dense dummy matmuls with no DMA deps ----
            scratch = ps.tile([128, 2, SQC], F32, name="warm", tag="st", bufs=2)
            for _ in range(PREWARM):
                nc.tensor.matmul(scratch[:, 0, 0:128], ident[:], ident[:],
                                 start=True, stop=True)

            # ---------------- emission helpers ----------------
            def emit_dmas_pre():
                # ordered for earliest compute start; sync queue is FIFO
                nc.sync.dma_start(wkv_t[:], wkv_il.ap())
                nc.sync.dma_start(wq_t[:, 0:4, :], wq_il[:, 0:4, :])
                nc.sync.dma_start(xfull[0][:, 0:1024], xT[0:128, 0:1024])
                nc.sync.dma_start(xfull[0][:, 1024:2048], xT[0:128, 1024:2048])
                nc.sync.dma_start(xfull[1][:], xT[128:256, :])
                nc.sync.dma_start(cos_t[:], cos4.ap())
                nc.sync.dma_start(sin_t[:], sin4.ap())
                for d in range(2, 6):
                    nc.sync.dma_start(xfull[d][:], xT[d * DC:(d + 1) * DC, :])
                nc.sync.dma_start(wq_t[:, 4:16, :], wq_il[:, 4:16, :])
                nc.sync.dma_start(mask_t[:], mask2.ap())
                for d in range(6, NDC):
                    nc.sync.dma_start(xfull[d][:], xT[d * DC:(d + 1) * DC, :])
                nc.sync.dma_start(wo_t[:], wo_il.ap())

            def rope_q(h, c0, qpair):
                """qpair: [128, 2, SQC] psum (or 2-tile list) -> qrot[h] cols
                [c0, c0+1024)."""
                qe = wpool.tile([128, 2, SQC], BF16, name="qe", tag="qe")
                if isinstance(qpair, list):
                    nc.vector.tensor_copy(qe[:, 0, :], qpair[0][:])
                    nc.vector.tensor_copy(qe[:, 1, :], qpair[1][:])
                else:
                    nc.vector.tensor_copy(qe[:], qpair[:])
                qef = qe[:].rearrange("p a b -> p (a b)")
                qsw = wpool.tile([128, 1024], BF16, name="qsw", tag="qsw")
                for g in range(4):
                    src = 32 * (g ^ 1)
                    nc.vector.tensor_copy(qsw[32 * g:32 * g + 32, :],
                                          qef[src:src + 32, :])
                nc.vector.tensor_mul(qef, qef, cos_t[:, c0:c0 + 1024])
                nc.vector.tensor_mul(qsw[:], qsw[:], sin_t[:, c0:c0 + 1024])
                nc.vector.tensor_add(qrot[h][:, c0:c0 + 1024], qef, qsw[:])

            def rope_kv(c0, kv0, kv1, jp):
                """K rope + V transpose for chunk pair at cols [c0, c0+1024)."""
                ke = wpool.tile([64, 1024], BF16, name="ke", tag="ke")
                nc.vector.tensor_copy(ke[:, 0:SQC], kv0[0:64, :])
                nc.vector.tensor_copy(ke[:, SQC:1024], kv1[0:64, :])
                vtmp = wpool.tile([64, 1024], BF16, name="vtmp", tag="vtmp")
                nc.scalar.copy(vtmp[:, 0:SQC], kv0[64:128, :])
                nc.scalar.copy(vtmp[:, SQC:1024], kv1[64:128, :])
                ksw = wpool.tile([64, 1024], BF16, name="ksw", tag="ksw")
                nc.vector.tensor_copy(ksw[0:32, :], ke[32:64, :])
                nc.vector.tensor_copy(ksw[32:64, :], ke[0:32, :])
                nc.vector.tensor_mul(ke[:], ke[:], cos_t[0:64, c0:c0 + 1024])
                nc.vector.tensor_mul(ksw[:], ksw[:], sin_t[0:64, c0:c0 + 1024])
                nc.vector.tensor_add(krot[0:64, c0:c0 + 1024], ke[:], ksw[:])
                nc.vector.tensor_copy(krot[64:128, c0:c0 + 1024],
                                      krot[0:64, c0:c0 + 1024])
                # V transpose: 8 PE transposes -> vaug chunks
                tps = [ps.tile([128, SQC], BF16, name=f"tps{j}", tag="proj")
                       for j in range(2)]
                for j in range(2):
                    for b in range(4):
                        i = 4 * (2 * jp + j) + b
                        dst = tps[j][:, 64 * b:64 * b + 64]
                        nc.tensor.transpose(dst, vtmp[:, (4 * j + b) * 128:
                                                      (4 * j + b) * 128 + 128],
                                            ident[0:64, 0:64])
                        nc.vector.tensor_copy(vaug[:, i, 0:DH], dst)

            def emit_A0():
                """jp=0: all 6 accumulators in parallel so the PE tracks the
                x DMA arrival; borrows the st tag (B hasn't started)."""
                s0, s1 = 0, SQC
                kv = [ps.tile([128, SQC], F32, name=f"kv{j}", tag="proj")
                      for j in range(2)]
                qt = [ps.tile([128, 2, SQC], F32, name=f"qtp{h}", tag="st",
                              bufs=2) for h in range(2)]
                for d in range(NDC):
                    st_, sp = (d == 0), (d == NDC - 1)
                    nc.tensor.matmul(kv[0][:], wkv_t[:, d, :],
                                     xfull[d][:, s0:s0 + SQC], start=st_, stop=sp)
                    nc.tensor.matmul(kv[1][:], wkv_t[:, d, :],
                                     xfull[d][:, s1:s1 + SQC], start=st_, stop=sp)
                    for h in range(2):
                        w = wq_t[:, d, 128 * h:128 * h + 128]
                        nc.tensor.matmul(qt[h][:, 0, :], w,
                                         xfull[d][:, s0:s0 + SQC],
                                         start=st_, stop=sp)
                        nc.tensor.matmul(qt[h][:, 1, :], w,
                                         xfull[d][:, s1:s1 + SQC],
                                         start=st_, stop=sp)
                rope_kv(0, kv[0], kv[1], 0)
                for h in range(2):
                    rope_q(h, 0, qt[h])

            def emit_A1():
                """jp=1: x resident; sequential pairs on the proj tag only
                (B(0,*) owns st by now and fills PE stalls)."""
                c0 = 1024
                s0, s1 = 2 * SQC, 3 * SQC
                kv = [ps.tile([128, SQC], F32, name=f"kv{j}", tag="proj")
                      for j in range(2)]
                for d in range(NDC):
                    st_, sp = (d == 0), (d == NDC - 1)
                    nc.tensor.matmul(kv[0][:], wkv_t[:, d, :],
                                     xfull[d][:, s0:s0 + SQC], start=st_, stop=sp)
                    nc.tensor.matmul(kv[1][:], wkv_t[:, d, :],
                                     xfull[d][:, s1:s1 + SQC], start=st_, stop=sp)
                rope_kv(c0, kv[0], kv[1], 1)
                for h in range(2):
                    qt = [ps.tile([128, SQC], F32, name=f"qt{h}{j}", tag="proj")
                          for j in range(2)]
                    for d in range(NDC):
                        st_, sp = (d == 0), (d == NDC - 1)
                        w = wq_t[:, d, 128 * h:128 * h + 128]
                        nc.tensor.matmul(qt[0][:], w, xfull[d][:, s0:s0 + SQC],
                                         start=st_, stop=sp)
                        nc.tensor.matmul(qt[1][:], w, xfull[d][:, s1:s1 + SQC],
                                         start=st_, stop=sp)
                    rope_q(h, c0, qt)

            def emit_B(hp, j):
                """Attention for head pair hp, sq chunk j."""
                s0 = j * SQC
                q = qrot[hp]
                av = [ps.tile([DH + 1, SQC], F32, name=f"av{h}", tag="av")
                      for h in range(2)]
                nsk_j = 4 * j + 4
                for i in range(nsk_j):
                    k0 = i * SKC
                    m = i - 4 * j
                    off = 0 if m < 1 else 128 * m
                    nw = SQC - off
                    st2 = ps.tile([128, 2, SQC], F32, name="st2", tag="st",
                                  bufs=2)
                    for h in range(2):
                        r0 = 64 * h
                        nc.tensor.matmul(st2[:, h, off:SQC],
                                         krot[r0:r0 + 64, k0:k0 + SKC],
                                         q[r0:r0 + 64, s0 + off:s0 + SQC],
                                         start=True, stop=True,
                                         tile_position=(r0, 0))
                    pt = wpool.tile([128, 2, SQC], BF16, name="pt", tag="pt",
                                    bufs=4)
                    nc.scalar.activation(pt[:, :, off:SQC], st2[:, :, off:SQC],
                                         EXP, scale=0.125)
                    if m >= 0:
                        nc.vector.tensor_mul(pt[:, :, off:SQC],
                                             pt[:, :, off:SQC],
                                             mask_t[:, m, :, off:SQC])
                    for h in range(2):
                        nc.tensor.matmul(av[h][:, off:SQC], vaug[:, i, :],
                                         pt[:, h, off:SQC],
                                         start=(i == 0), stop=(i == nsk_j - 1))

                # evacuate AV to SBUF right away (frees the av banks; the
                # softmax normalize then runs off the critical PSUM path)
                avf = wpool.tile([DH + 1, 2, SQC], F32, name="avf", tag="avf")
                for h in range(2):
                    nc.vector.tensor_copy(avf[:, h, :], av[h][:])

                # softmax normalize: 1/Z broadcast, write attnT
                for h in range(2):
                    nc.vector.tensor_copy(zg[32 * h:32 * h + 1, :],
                                          avf[64:65, h, :])
                nc.vector.reciprocal_approx_fast(zr[0:33, :], zg[0:33, :])
                for h in range(2):
                    if h == 0:
                        zsrc = zr[0:1, :]
                    else:
                        nc.vector.tensor_copy(z0[:], zr[32:33, :])
                        zsrc = z0[:]
                    bc = wpool.tile([64, SQC], F32, name="bc", tag="bc")
                    nc.gpsimd.partition_broadcast(bc[:], zsrc)
                    nc.vector.tensor_mul(attnT[hp][64 * h:64 * h + 64, s0:s0 + SQC],
                                         avf[0:64, h, :], bc[:])

            def emit_C(si):
                """Output projection for seq rows [128si, 128si+128)."""
                stage = wpool.tile([128, D], BF16, name="cstage", tag="cstage")
                for oi in range(4):
                    o0 = oi * SQC
                    cps = ps.tile([128, SQC], F32, name="cps", tag="proj")
                    for t in range(2):
                        nc.tensor.matmul(cps[:],
                                         attnT[t][:, si * 128:(si + 1) * 128],
                                         wo_t[:, t, o0:o0 + SQC],
                                         start=(t == 0), stop=(t == 1))
                    nc.vector.tensor_copy(stage[:, o0:o0 + SQC], cps[:])
                nc.gpsimd.dma_start(out[si * 128:(si + 1) * 128, :], stage[:])

            # ---------------- the interleaved pipeline ----------------
            emit_dmas_pre()
            emit_A0()
            emit_B(0, 0)
            emit_B(0, 1)
            emit_A1()
            emit_B(1, 0)
            emit_B(1, 1)
            for si in range(0, 4):
                emit_C(si)
            emit_B(0, 2)
            emit_B(0, 3)
            for si in range(4, 8):
                emit_C(si)
            emit_B(1, 2)
            emit_B(1, 3)
            for si in range(8, 16):
                emit_C(si)

    nc.compile()
    _PROGRAM_CACHE["nc"] = nc
    return nc


def prep_in_maps(x, freqs_cos, freqs_sin, wq, wk, wv, wo):
    """Host-side sharding / pre-transposition. Returns list of 8 in_maps."""
    import ml_dtypes
    mmd = ml_dtypes.bfloat16

    x = np.asarray(x, dtype=np.float32)
    freqs_cos = np.asarray(freqs_cos, dtype=np.float32)
    freqs_sin = np.asarray(freqs_sin, dtype=np.float32)
    wq = np.asarray(wq, dtype=np.float32)
    wk = np.asarray(wk, dtype=np.float32)
    wv = np.asarray(wv, dtype=np.float32)
    wo = np.asarray(wo, dtype=np.float32)

    xT = np.ascontiguousarray(x.reshape(S, D).T).astype(mmd)   # [D, S]

    # head-dim permutation: even lanes first, odd lanes second
    perm = np.concatenate([np.arange(0, DH, 2), np.arange(1, DH, 2)])
    wq_h = wq.reshape(NH, DH, D)[:, perm, :]               # [NH, DH, D]
    wk_h = wk.reshape(NKV, DH, D)[:, perm, :]              # [NKV, DH, D]
    wv_h = wv.reshape(NKV, DH, D)                          # not permuted

    # cos rows tiled x4; sin rows: [-sin; +sin] tiled x2 (signs baked in)
    cosT = np.ascontiguousarray(freqs_cos.T)               # [32, S]
    sinT = np.ascontiguousarray(freqs_sin.T)
    cos4 = np.ascontiguousarray(np.tile(cosT, (4, 1))).astype(mmd)
    sin4 = np.ascontiguousarray(
        np.tile(np.concatenate([-sinT, sinT], axis=0), (2, 1))).astype(mmd)

    # causal masks for 4 diagonal offsets, duplicated for the 2 heads of a
    # pair: mask2[p, m, h, f] = f >= p + 128m
    p_idx = np.arange(128)[:, None, None, None]
    m_idx = np.arange(4)[None, :, None, None]
    f_idx = np.arange(SQC)[None, None, None, :]
    mask2 = np.broadcast_to((f_idx >= p_idx + 128 * m_idx),
                            (128, 4, 2, SQC)).astype(mmd)
    mask2 = np.ascontiguousarray(mask2)

    in_maps = []
    for c in range(NCORES):
        wq_c = wq_h[HQ * c:HQ * (c + 1)].reshape(HQ * DH, D)   # [256, D]
        wqT_c = np.ascontiguousarray(wq_c.T).astype(mmd)       # [D, 256]
        wq_int = np.ascontiguousarray(
            wqT_c.reshape(NDC, 128, HQ * DH).transpose(1, 0, 2))
        wkv_c = np.concatenate([wk_h[c], wv_h[c]], axis=0)     # [128, D]
        wkvT_c = np.ascontiguousarray(wkv_c.T).astype(mmd)     # [D, 128]
        wkv_int = np.ascontiguousarray(
            wkvT_c.reshape(NDC, 128, 2 * DH).transpose(1, 0, 2))
        woT_c = np.ascontiguousarray(
            wo[:, HQ * DH * c:HQ * DH * (c + 1)].T).astype(mmd)  # [256, D]
        wo_int = np.ascontiguousarray(
            woT_c.reshape(2, 128, D).transpose(1, 0, 2))
        in_maps.append({
            "xT": xT, "wq_il": wq_int, "wkv_il": wkv_int, "wo_il": wo_int,
            "cos4": cos4, "sin4": sin4, "mask2": mask2,
        })
    return in_maps


def run(inputs, trace=False, trace_cores=None, tmpdir=None):
    """Compile (cached), run on 8 cores, gather. Returns (output, results)."""
    nc = build_program()
    in_maps = prep_in_maps(**inputs)
    res = run_bass_kernel_spmd(nc, in_maps, core_ids=list(range(NCORES)),
                               trace=trace, trace_cores=trace_cores,
                               tmpdir=tmpdir)
    acc = np.zeros((S, D), dtype=np.float32)
    for r in res.results:
        acc += r["out"].astype(np.float32)
    return acc.reshape(1, S, D), res


def kernel(**inputs):
    out, _ = run(inputs)
    return out
